# revision 25
# baseline (speedup 1.0000x reference)
"""Causal multi-head attention block on 8 Trainium2 NeuronCores.

Problem: B=2, T=4096, C=128, H=4, Dh=32 (fp32).
  qkv = x @ qkv_w.T + qkv_b ; causal softmax attention ; y = out @ out_w.T + out_b

Sharding: 8 cores = (batch B=2) x (heads H=4). Each core owns one (b, h)
pair end to end. The device returns the *unnormalized* head output
yT[h] = (P @ V) @ Wo_h.T (transposed, [C, T]) plus the softmax row-sums;
the host divides by the row-sums, sums the 4 head partials per batch, and
adds out_b. Softmax normalization commutes with the linear maps -> exact.

On-device design (per core). The cost model charges matmuls by streamed
(moving) columns only, so P@V runs with the P^T block as the stationary
operand: o[128q, 33] += pt[128k, 128q]^T @ vaug[128k, 33] streams just 33
columns per (key-tile, query-subtile) block. pt and vaug are bf16 (full PE
rate at any width; P/v quantization ~0.4% each, well inside the 2e-2 rel
budget). o is transposed back on the PE (bf16 identity) for the output
projection; the softmax row-sum rides the PV accumulation as a ones column.

exp is the throughput limit (ACT 0.833 ns/elem/lane), so roughly half the
off-diagonal exps run on the DVE as a one-instruction Schraudolph exp:
i16 = rne_sat(s * 128/ln2 + (127*128 + c)) bitcast to bf16 gives exp(s)
within ~3.3% (final output err ~4e-3, validated in numpy + on HW). The
diagonal (causally masked, -1e9 biased) blocks stay on ACT's exact exp.
Engine assignment of exps and PSUM evacuations is a greedy static load
balance between ACT and DVE (GPSIMD cannot touch PSUM; it only triggers
SWDGE DMAs).
"""

import math
import os
from contextlib import ExitStack

import numpy as np

import concourse.bass as bass
import concourse.tile as tile
from concourse import bacc, mybir
from concourse.bass_utils import run_bass_kernel_spmd

B, T, C = 2, 4096, 128
H, DH = 4, 32
NCORES = 8
TQ = 512          # query block per group
NG = T // TQ      # 8 query groups
F32 = mybir.dt.float32
F32R = mybir.dt.float32r
BF16 = mybir.dt.bfloat16
I16 = mybir.dt.int16

# Schraudolph bf16 exp constants: i16 = rne(s*A16 + B16), bitcast bf16
A16 = 128.0 / math.log(2.0)
B16 = 127.0 * 128.0 - 5.5

_CACHE = {}
last_exec_time_ns = None
last_results = None


def round_fp32r(a):
    """Round fp32 to fp32r (drop low 12 mantissa bits, round-to-nearest-even)."""
    u = np.ascontiguousarray(a, dtype=np.float32).view(np.uint32)
    low = u & np.uint32(0xFFF)
    base = u & np.uint32(0xFFFFF000)
    up = (low > 0x800) | ((low == 0x800) & (((base >> np.uint32(12)) & np.uint32(1)) == 1))
    return (base + (up.astype(np.uint32) << np.uint32(12))).view(np.float32)


def build_program():
    if "nc" in _CACHE:
        return _CACHE["nc"]
    nc = bacc.Bacc(
        "TRN2",
        target_bir_lowering=False,
        debug=False,
        enable_asserts=False,
        num_devices=NCORES,
    )
    xt = nc.dram_tensor("xt", [C, T], F32R, kind="ExternalInput").ap()
    xtb = nc.dram_tensor("xtb", [C, T], BF16, kind="ExternalInput").ap()
    # wconst: wqk [:, 0:64] and the q'/k bias column [0:64, 64:65]
    wconst = nc.dram_tensor("wconst", [C, 80], F32R, kind="ExternalInput").ap()
    wv = nc.dram_tensor("wv", [C, DH], BF16, kind="ExternalInput").ap()
    bv = nc.dram_tensor("bv", [1, DH + 1], BF16, kind="ExternalInput").ap()
    wo = nc.dram_tensor("wo", [DH, C], F32R, kind="ExternalInput").ap()
    # mconst (bf16): Wm [128,128] (Wm[m,x] = -1e9*[m==x+1]; A^T@Wm = causal
    # -1e9 for key jj > query x), A = tril ones, I = identity, ones row.
    mconst = nc.dram_tensor("mconst", [128, 512], BF16, kind="ExternalInput").ap()
    yt = nc.dram_tensor("yt", [C, T], F32, kind="ExternalOutput").ap()
    sums = nc.dram_tensor("sums", [1, T], F32, kind="ExternalOutput").ap()

    # greedy static ACT/DVE load balance (ns estimates incl fixed overheads)
    load = {"act": 0.0, "dve": 0.0}

    def pick(act_cost, dve_cost):
        eng = "act" if load["act"] + act_cost <= load["dve"] + dve_cost else "dve"
        load[eng] += act_cost if eng == "act" else dve_cost
        return eng

    with ExitStack() as ctx:
        tc = ctx.enter_context(tile.TileContext(nc))
        const = ctx.enter_context(tc.tile_pool(name="const", bufs=1))
        pool_p = ctx.enter_context(tc.tile_pool(name="pT", bufs=6))
        pool_ot = ctx.enter_context(tc.tile_pool(name="ot", bufs=3))
        pool_t = ctx.enter_context(tc.tile_pool(name="oT", bufs=2))
        pool_y = ctx.enter_context(tc.tile_pool(name="yt", bufs=3))
        # PSUM budget (8 banks): psS 3x[128,1024] = 6 (3 S-pair slots so the
        # S(q) -> exp(q) -> S(q+3) ring never throttles the exp engines),
        # psO [128,132] = 1, psA 1 (projections + oT transpose + y rotate).
        ps_s = ctx.enter_context(tc.tile_pool(name="psS", bufs=3, space="PSUM"))
        ps_o = ctx.enter_context(tc.tile_pool(name="psO", bufs=1, space="PSUM"))
        ps_a = ctx.enter_context(tc.tile_pool(name="psA", bufs=1, space="PSUM"))

        s_wc = const.tile([C, 80], F32R)
        s_wv = const.tile([C, DH], BF16)
        s_bv = const.tile([1, DH + 1], BF16)
        s_wo = const.tile([DH, C], F32R)
        s_mc = const.tile([128, 512], BF16)
        s_xts = [const.tile([C, TQ], F32R, name=f"xt{c}") for c in range(NG)]
        s_xbs = [const.tile([C, TQ], BF16, name=f"xb{c}") for c in range(NG)]
        s_qts = [const.tile([DH, TQ], F32R, name=f"qt{c}") for c in range(NG)]
        s_kts = [const.tile([DH, TQ], F32R, name=f"kt{c}") for c in range(NG)]
        s_vas = [const.tile([128, 4 * (DH + 1)], BF16, name=f"va{c}") for c in range(NG)]

        # critical-path DMAs first, split across two parallel DMA lanes
        # (sync -> HWDGE; gpsimd -> SWDGE on the otherwise idle Q7 cores)
        nc.sync.dma_start(out=s_wc, in_=wconst)
        for c in range(4):
            nc.sync.dma_start(out=s_xts[c], in_=xt[:, c * TQ : (c + 1) * TQ])
        nc.gpsimd.dma_start(out=s_mc, in_=mconst)
        nc.gpsimd.dma_start(out=s_wv, in_=wv)
        nc.gpsimd.dma_start(out=s_bv, in_=bv)
        nc.gpsimd.dma_start(out=s_xbs[0], in_=xtb[:, 0:TQ])
        for c in range(4, NG):
            nc.gpsimd.dma_start(out=s_xts[c], in_=xt[:, c * TQ : (c + 1) * TQ])
        for c in range(1, NG):
            nc.gpsimd.dma_start(out=s_xbs[c], in_=xtb[:, c * TQ : (c + 1) * TQ])
        nc.gpsimd.dma_start(out=s_wo, in_=wo)

        s_wqk = s_wc[:, 0:64]
        s_bq = s_wc[0:DH, 64:65].bitcast(F32)
        s_bk = s_wc[DH : 2 * DH, 64:65].bitcast(F32)
        s_Wm = s_mc[:, 0:128]
        s_A = s_mc[:, 128:256]
        s_I = s_mc[:, 256:384]
        s_ones = s_mc[0:1, 384:512]

        def qk_proj_chunk(c):
            p_qk = ps_a.tile([64, TQ], F32, tag="ps_main")
            nc.tensor.matmul(
                out=p_qk, lhsT=s_wqk, rhs=s_xts[c], start=True, stop=True
            )
            # q' evac (bias applied per-partition); alternate engines
            if pick(612, 784) == "act":
                nc.scalar.activation(
                    out=s_qts[c], in_=p_qk[0:DH, :],
                    func=mybir.ActivationFunctionType.Identity, bias=s_bq,
                )
            else:
                nc.vector.tensor_scalar_add(s_qts[c], p_qk[0:DH, :], s_bq)
            if pick(612, 784) == "act":
                nc.scalar.activation(
                    out=s_kts[c], in_=p_qk[DH : 2 * DH, :],
                    func=mybir.ActivationFunctionType.Identity, bias=s_bk,
                )
            else:
                nc.vector.tensor_scalar_add(s_kts[c], p_qk[DH : 2 * DH, :], s_bk)

        # v projection for one 512-chunk (4 key tiles) in bf16, one PSUM bank,
        # ones column appended via the bias-row matmul (has_written semantics).
        def v_proj_chunk(c):
            p_v = ps_a.tile([128, 4 * (DH + 1)], F32, tag="ps_main")
            for r in range(4):
                c0 = r * (DH + 1)
                nc.tensor.matmul(
                    out=p_v[:, c0 : c0 + DH],
                    lhsT=s_xbs[c][:, r * 128 : (r + 1) * 128],
                    rhs=s_wv,
                    start=(r == 0), stop=False,
                )
                nc.tensor.matmul(
                    out=p_v[:, c0 : c0 + DH + 1], lhsT=s_ones, rhs=s_bv,
                    start=False, stop=(r == 3),
                )
            if pick(295, 262) == "act":
                nc.scalar.activation(
                    out=s_vas[c], in_=p_v,
                    func=mybir.ActivationFunctionType.Identity,
                )
            else:
                nc.vector.tensor_copy(out=s_vas[c], in_=p_v)

        def q_of(g):
            return s_qts[g]

        def k_of(j):
            return s_kts[j // 4][:, (j % 4) * 128 : (j % 4 + 1) * 128]

        def v_of(j):
            c0 = (j % 4) * (DH + 1)
            return s_vas[j // 4][:, c0 : c0 + DH + 1]

        def emit_exp(pt, p_st, width, eng):
            if eng == "act":
                nc.scalar.activation(
                    out=pt[:, 0:width], in_=p_st,
                    func=mybir.ActivationFunctionType.Exp,
                )
            else:
                nc.vector.tensor_scalar(
                    out=pt[:, 0:width].bitcast(I16), in0=p_st,
                    scalar1=A16, scalar2=B16,
                    op0=mybir.AluOpType.mult, op1=mybir.AluOpType.add,
                )

        qk_proj_chunk(0)
        qk_proj_chunk(1)

        # deferred-work FIFO: PV flushes, accumulator evacs, and group tails
        # are emitted ~LAG pair-slots behind their exp so the in-order PE
        # stream always has independent S matmuls to chew on while ACT/DVE
        # run the exps.
        work_q = []
        LAG = 2

        def drain(to_len):
            while len(work_q) > to_len:
                work_q.pop(0)()

        # Process the biggest groups mid-flight and end on a small one so the
        # final serial tail (exp -> PV -> evac -> transpose -> proj -> DMA)
        # sits behind a short group. Projections are emitted just-in-time.
        ORDER = [0, 1, 7, 6, 5, 4, 3, 2]
        done_qk = {0, 1}
        done_v = set()

        def ensure_projs(kq_need, v_need):
            for c in range(max(kq_need, v_need) + 1):
                if c <= kq_need and c not in done_qk:
                    done_qk.add(c)
                    qk_proj_chunk(c)
                if c <= v_need and c not in done_v:
                    done_v.add(c)
                    v_proj_chunk(c)

        for si, g in enumerate(ORDER):
            i0 = g * TQ
            # PV accumulation: start resets the whole PSUM bank, so only the
            # group's first block starts and only its last block stops;
            # disjoint 33-col regions rely on has_written semantics.
            pv_cnt = [0]
            pv_tot = 16 * g + 10
            p_acc = ps_o.tile([128, 4 * (DH + 1)], F32, tag="ps_acc")

            def pv_block(pt_slice, j, s, p_acc=p_acc, pv_cnt=pv_cnt, pv_tot=pv_tot):
                c0 = s * (DH + 1)
                nc.tensor.matmul(
                    out=p_acc[:, c0 : c0 + DH + 1],
                    lhsT=pt_slice,
                    rhs=v_of(j),
                    start=(pv_cnt[0] == 0),
                    stop=(pv_cnt[0] + 1 == pv_tot),
                )
                pv_cnt[0] += 1

            # off-diagonal key tiles in pairs: one exp per [128, 1024]
            for q in range(2 * g):
                j0 = 2 * q
                # just-in-time projections: S of this pair needs its k chunk
                # (stay one ahead); queued PV flushes drain LAG behind and
                # need their v chunks by then. Slot 1 pre-stages the rest of
                # the qk chunks so the out-of-order groups find their own q
                # chunk ready (slot 2 runs group 7).
                kq = (2 * q + 3) // 4 + 1
                if si == 1:
                    kq = max(kq, 5 + 2 * q)
                ensure_projs(min(NG - 1, kq),
                             min(g, (2 * q + 2 * LAG + 1) // 4))
                p_st = ps_s.tile([128, 2 * TQ], F32, tag="ps_st")
                for u in range(2):
                    nc.tensor.matmul(
                        out=p_st[:, u * TQ : (u + 1) * TQ],
                        lhsT=k_of(j0 + u),
                        rhs=q_of(g),
                        start=True, stop=True,
                    )
                pt = pool_p.tile([128, 2 * TQ], BF16, tag="pt")
                emit_exp(pt, p_st, 2 * TQ, pick(1038, 1192))

                def pv_pair(pt=pt, j0=j0, pv_block=pv_block):
                    for u in range(2):
                        for s in range(4):
                            pv_block(pt[:, u * TQ + 128 * s : u * TQ + 128 * (s + 1)],
                                     j0 + u, s)

                work_q.append(pv_pair)
                drain(LAG)

            # diagonal tiles r=0..3 cover queries [128r, 512); causal -1e9
            # bias added by the A^T@Wm matmul on the 128 columns next to the
            # diagonal. T1 = [r0 512 | r1 384], T2 = [r2 256 | r3 128].
            ensure_projs(max(g, 3) if si == 0 else g, g)
            p1 = ps_s.tile([128, 896], F32, tag="ps_st")
            nc.tensor.matmul(out=p1[:, 0:512], lhsT=k_of(4 * g), rhs=q_of(g),
                             start=True, stop=False)
            nc.tensor.matmul(out=p1[:, 0:128], lhsT=s_A, rhs=s_Wm,
                             start=False, stop=True)
            nc.tensor.matmul(out=p1[:, 512:896], lhsT=k_of(4 * g + 1),
                             rhs=q_of(g)[:, 128:512], start=True, stop=False)
            nc.tensor.matmul(out=p1[:, 512:640], lhsT=s_A, rhs=s_Wm,
                             start=False, stop=True)
            pt1 = pool_p.tile([128, 2 * TQ], BF16, tag="pt")
            emit_exp(pt1, p1, 896, "act")
            load["act"] += 931

            def pv_diag1(pt1=pt1, g=g, pv_block=pv_block):
                for s in range(4):
                    pv_block(pt1[:, 128 * s : 128 * (s + 1)], 4 * g, s)
                for s in range(1, 4):
                    pv_block(pt1[:, 512 + 128 * (s - 1) : 512 + 128 * s], 4 * g + 1, s)

            work_q.append(pv_diag1)
            drain(LAG)

            p2 = ps_s.tile([128, 384], F32, tag="ps_st")
            nc.tensor.matmul(out=p2[:, 0:256], lhsT=k_of(4 * g + 2),
                             rhs=q_of(g)[:, 256:512], start=True, stop=False)
            nc.tensor.matmul(out=p2[:, 0:128], lhsT=s_A, rhs=s_Wm,
                             start=False, stop=False)
            nc.tensor.matmul(out=p2[:, 256:384], lhsT=k_of(4 * g + 3),
                             rhs=q_of(g)[:, 384:512], start=False, stop=False)
            nc.tensor.matmul(out=p2[:, 256:384], lhsT=s_A, rhs=s_Wm,
                             start=False, stop=True)
            pt2 = pool_p.tile([128, 2 * TQ], BF16, tag="pt")
            emit_exp(pt2, p2, 384, "act")
            load["act"] += 505
            s_ob = pool_ot.tile([128, 4 * (DH + 1)], BF16, tag="ot")

            def pv_diag2_and_evac(pt2=pt2, g=g, s_ob=s_ob, p_acc=p_acc,
                                  pv_block=pv_block):
                for s in range(2, 4):
                    pv_block(pt2[:, 128 * (s - 2) : 128 * (s - 1)], 4 * g + 2, s)
                pv_block(pt2[:, 256:384], 4 * g + 3, 3)
                # evacuate the PV accumulator (frees psO for the next group)
                if pick(295, 262) == "act":
                    nc.scalar.activation(
                        out=s_ob, in_=p_acc,
                        func=mybir.ActivationFunctionType.Identity,
                    )
                else:
                    nc.vector.tensor_copy(out=s_ob, in_=p_acc)

            work_q.append(pv_diag2_and_evac)

            def tail(s_ob=s_ob, i0=i0):
                # transpose o [128q, 33] subtiles -> oT [33, 512] on the PE
                p_t = ps_a.tile([DH + 1, TQ], BF16, tag="ps_main")
                for s in range(4):
                    c0 = s * (DH + 1)
                    nc.tensor.matmul(
                        out=p_t[:, 128 * s : 128 * (s + 1)],
                        lhsT=s_ob[:, c0 : c0 + DH + 1],
                        rhs=s_I, is_transpose=True,
                        start=(s == 0), stop=(s == 3),
                    )
                s_ot = pool_t.tile([DH + 1, TQ], F32R, tag="oT")
                if pick(612, 783) == "act":
                    nc.scalar.activation(
                        out=s_ot, in_=p_t,
                        func=mybir.ActivationFunctionType.Identity,
                    )
                else:
                    nc.vector.tensor_copy(out=s_ot, in_=p_t)
                p_y = ps_a.tile([C, TQ], F32, tag="ps_main")
                nc.tensor.matmul(
                    out=p_y, lhsT=s_wo, rhs=s_ot[0:DH, :], start=True, stop=True
                )
                s_y = pool_y.tile([C, TQ], F32, tag="y")
                if pick(612, 783) == "act":
                    nc.scalar.activation(
                        out=s_y, in_=p_y,
                        func=mybir.ActivationFunctionType.Identity,
                    )
                else:
                    nc.vector.tensor_copy(out=s_y, in_=p_y)
                nc.sync.dma_start(out=yt[:, i0 : i0 + TQ], in_=s_y)
                nc.sync.dma_start(
                    out=sums[:, i0 : i0 + TQ],
                    in_=s_ot[DH : DH + 1, :].bitcast(F32),
                )

            work_q.append(tail)

        drain(0)

    nc.compile()
    _CACHE["nc"] = nc
    return nc


def _host_inputs(x, qkv_w, qkv_b, out_w, out_b):
    import ml_dtypes

    scale = 1.0 / math.sqrt(DH)
    mm = np.arange(128)[:, None]
    w_blk = -1e9 * (mm == np.arange(128)[None, :] + 1).astype(np.float32)
    a_blk = (mm <= np.arange(128)[None, :]).astype(np.float32)
    i_blk = np.eye(128, dtype=np.float32)
    ones_blk = np.zeros((128, 128), dtype=np.float32)
    ones_blk[0, :] = 1.0
    mconst = np.concatenate([w_blk, a_blk, i_blk, ones_blk], axis=1).astype(
        ml_dtypes.bfloat16
    )
    in_maps = []
    for c in range(NCORES):
        b, h = c // 4, c % 4
        wq = qkv_w[h * DH : (h + 1) * DH, :] * scale          # [32, 128]
        wk = qkv_w[C + h * DH : C + (h + 1) * DH, :]
        wv_ = qkv_w[2 * C + h * DH : 2 * C + (h + 1) * DH, :]
        bq = qkv_b[h * DH : (h + 1) * DH] * scale
        bk = qkv_b[C + h * DH : C + (h + 1) * DH]
        bv_ = qkv_b[2 * C + h * DH : 2 * C + (h + 1) * DH]
        wconst = np.zeros((C, 80), dtype=np.float32)
        wconst[:, 0:64] = np.concatenate([wq, wk], axis=0).T
        wconst[0:64, 64] = np.concatenate([bq, bk])
        in_maps.append(
            {
                "xt": round_fp32r(x[b].T),
                "xtb": np.ascontiguousarray(x[b].T).astype(ml_dtypes.bfloat16),
                "wconst": round_fp32r(wconst),
                "wv": wv_.T.astype(ml_dtypes.bfloat16),
                "bv": np.concatenate([bv_, [1.0]]).astype(ml_dtypes.bfloat16)[None, :],
                "wo": round_fp32r(out_w[:, h * DH : (h + 1) * DH].T),
                "mconst": np.ascontiguousarray(mconst),
            }
        )
    return in_maps


def kernel(x, qkv_w, qkv_b, out_w, out_b):
    global last_exec_time_ns, last_results
    x = np.asarray(x, dtype=np.float32)
    qkv_w = np.asarray(qkv_w, dtype=np.float32)
    qkv_b = np.asarray(qkv_b, dtype=np.float32)
    out_w = np.asarray(out_w, dtype=np.float32)
    out_b = np.asarray(out_b, dtype=np.float32)

    nc = build_program()
    in_maps = _host_inputs(x, qkv_w, qkv_b, out_w, out_b)
    try:
        res = run_bass_kernel_spmd(
            nc,
            in_maps,
            list(range(NCORES)),
            trace=bool(int(os.environ.get("KERNEL_TRACE", "0"))),
        )
    except ModuleNotFoundError:
        os.environ["BASS_NEVER_TRACE"] = "1"
        res = run_bass_kernel_spmd(nc, in_maps, list(range(NCORES)), trace=False)
    last_results = res
    last_exec_time_ns = res.exec_time_ns

    y = np.empty((B, T, C), dtype=np.float32)
    for b in range(B):
        acc = np.zeros((C, T), dtype=np.float32)
        for h in range(H):
            r = res.results[b * 4 + h]
            acc += r["yt"] / r["sums"]
        y[b] = acc.T + out_b[None, :]
    return y


# revision 28
# speedup vs baseline: 1.0258x; 1.0258x over previous
"""Causal multi-head attention block on 8 Trainium2 NeuronCores.

Problem: B=2, T=4096, C=128, H=4, Dh=32 (fp32).
  qkv = x @ qkv_w.T + qkv_b ; causal softmax attention ; y = out @ out_w.T + out_b

Sharding: 8 cores = (batch B=2) x (heads H=4). Each core owns one (b, h)
pair end to end. The device returns the *unnormalized* head output
yT[h] = (P @ V) @ Wo_h.T (transposed, [C, T]) plus the softmax row-sums;
the host divides by the row-sums, sums the 4 head partials per batch, and
adds out_b. Softmax normalization commutes with the linear maps -> exact.

On-device design (per core). The cost model charges matmuls by streamed
(moving) columns only, so P@V runs with the P^T block as the stationary
operand: o[128q, 33] += pt[128k, 128q]^T @ vaug[128k, 33] streams just 33
columns per (key-tile, query-subtile) block. pt and vaug are bf16 (full PE
rate at any width; P/v quantization ~0.4% each, well inside the 2e-2 rel
budget). o is transposed back on the PE (bf16 identity) for the output
projection; the softmax row-sum rides the PV accumulation as a ones column.

exp is the throughput limit (ACT 0.833 ns/elem/lane), so roughly half the
off-diagonal exps run on the DVE as a one-instruction Schraudolph exp:
i16 = rne_sat(s * 128/ln2 + (127*128 + c)) bitcast to bf16 gives exp(s)
within ~3.3% (final output err ~4e-3, validated in numpy + on HW). The
diagonal (causally masked, -1e9 biased) blocks stay on ACT's exact exp.
Engine assignment of exps and PSUM evacuations is a greedy static load
balance between ACT and DVE (GPSIMD cannot touch PSUM; it only triggers
SWDGE DMAs).
"""

import math
import os
from contextlib import ExitStack

import numpy as np

import concourse.bass as bass
import concourse.tile as tile
from concourse import bacc, mybir
from concourse.bass_utils import run_bass_kernel_spmd

B, T, C = 2, 4096, 128
H, DH = 4, 32
NCORES = 8
TQ = 512          # query block per group
NG = T // TQ      # 8 query groups
F32 = mybir.dt.float32
F32R = mybir.dt.float32r
BF16 = mybir.dt.bfloat16
I16 = mybir.dt.int16

# Schraudolph bf16 exp constants: i16 = rne(s*A16 + B16), bitcast bf16
A16 = 128.0 / math.log(2.0)
B16 = 127.0 * 128.0 - 5.5

_CACHE = {}
last_exec_time_ns = None
last_results = None


def round_fp32r(a):
    """Round fp32 to fp32r (drop low 12 mantissa bits, round-to-nearest-even)."""
    u = np.ascontiguousarray(a, dtype=np.float32).view(np.uint32)
    low = u & np.uint32(0xFFF)
    base = u & np.uint32(0xFFFFF000)
    up = (low > 0x800) | ((low == 0x800) & (((base >> np.uint32(12)) & np.uint32(1)) == 1))
    return (base + (up.astype(np.uint32) << np.uint32(12))).view(np.float32)


def build_program():
    if "nc" in _CACHE:
        return _CACHE["nc"]
    nc = bacc.Bacc(
        "TRN2",
        target_bir_lowering=False,
        debug=False,
        enable_asserts=False,
        num_devices=NCORES,
    )
    xt = nc.dram_tensor("xt", [C, T], F32R, kind="ExternalInput").ap()
    xtb = nc.dram_tensor("xtb", [C, T], BF16, kind="ExternalInput").ap()
    # wconst: wqk [:, 0:64] and the q'/k bias column [0:64, 64:65]
    wconst = nc.dram_tensor("wconst", [C, 80], F32R, kind="ExternalInput").ap()
    wv = nc.dram_tensor("wv", [C, DH], BF16, kind="ExternalInput").ap()
    bv = nc.dram_tensor("bv", [1, DH + 1], BF16, kind="ExternalInput").ap()
    wo = nc.dram_tensor("wo", [DH, C], F32R, kind="ExternalInput").ap()
    # mconst (bf16): Wm [128,128] (Wm[m,x] = -1e9*[m==x+1]; A^T@Wm = causal
    # -1e9 for key jj > query x), A = tril ones, I = identity, ones row.
    mconst = nc.dram_tensor("mconst", [128, 512], BF16, kind="ExternalInput").ap()
    yt = nc.dram_tensor("yt", [C, T], F32, kind="ExternalOutput").ap()
    sums = nc.dram_tensor("sums", [1, T], F32, kind="ExternalOutput").ap()

    # greedy static ACT/DVE load balance (ns estimates incl fixed overheads)
    load = {"act": 0.0, "dve": 0.0}

    def pick(act_cost, dve_cost):
        eng = "act" if load["act"] + act_cost <= load["dve"] + dve_cost else "dve"
        load[eng] += act_cost if eng == "act" else dve_cost
        return eng

    with ExitStack() as ctx:
        tc = ctx.enter_context(tile.TileContext(nc))
        const = ctx.enter_context(tc.tile_pool(name="const", bufs=1))
        pool_p = ctx.enter_context(tc.tile_pool(name="pT", bufs=6))
        pool_ot = ctx.enter_context(tc.tile_pool(name="ot", bufs=3))
        pool_t = ctx.enter_context(tc.tile_pool(name="oT", bufs=2))
        pool_y = ctx.enter_context(tc.tile_pool(name="yt", bufs=3))
        # PSUM budget (8 banks): psS 3x[128,1024] = 6 (3 S-pair slots so the
        # S(q) -> exp(q) -> S(q+3) ring never throttles the exp engines),
        # psO [128,132] = 1, psA 1 (projections + oT transpose + y rotate).
        ps_s = ctx.enter_context(tc.tile_pool(name="psS", bufs=3, space="PSUM"))
        ps_o = ctx.enter_context(tc.tile_pool(name="psO", bufs=1, space="PSUM"))
        ps_a = ctx.enter_context(tc.tile_pool(name="psA", bufs=1, space="PSUM"))

        s_wc = const.tile([C, 80], F32R)
        s_wv = const.tile([C, DH], BF16)
        s_bv = const.tile([1, DH + 1], BF16)
        s_wo = const.tile([DH, C], F32R)
        s_mc = const.tile([128, 512], BF16)
        s_xts = [const.tile([C, TQ], F32R, name=f"xt{c}") for c in range(NG)]
        s_xbs = [const.tile([C, TQ], BF16, name=f"xb{c}") for c in range(NG)]
        s_qts = [const.tile([DH, TQ], F32R, name=f"qt{c}") for c in range(NG)]
        s_kts = [const.tile([DH, TQ], F32R, name=f"kt{c}") for c in range(NG)]
        s_vas = [const.tile([128, 4 * (DH + 1)], BF16, name=f"va{c}") for c in range(NG)]

        # critical-path DMAs first, split across two parallel DMA lanes
        # (sync -> HWDGE; gpsimd -> SWDGE on the otherwise idle Q7 cores)
        nc.sync.dma_start(out=s_wc, in_=wconst)
        for c in range(4):
            nc.sync.dma_start(out=s_xts[c], in_=xt[:, c * TQ : (c + 1) * TQ])
        nc.gpsimd.dma_start(out=s_mc, in_=mconst)
        nc.gpsimd.dma_start(out=s_wv, in_=wv)
        nc.gpsimd.dma_start(out=s_bv, in_=bv)
        nc.gpsimd.dma_start(out=s_xbs[0], in_=xtb[:, 0:TQ])
        for c in range(4, NG):
            nc.gpsimd.dma_start(out=s_xts[c], in_=xt[:, c * TQ : (c + 1) * TQ])
        for c in range(1, NG):
            nc.gpsimd.dma_start(out=s_xbs[c], in_=xtb[:, c * TQ : (c + 1) * TQ])
        nc.gpsimd.dma_start(out=s_wo, in_=wo)

        s_wqk = s_wc[:, 0:64]
        s_bq = s_wc[0:DH, 64:65].bitcast(F32)
        s_bk = s_wc[DH : 2 * DH, 64:65].bitcast(F32)
        s_Wm = s_mc[:, 0:128]
        s_A = s_mc[:, 128:256]
        s_I = s_mc[:, 256:384]
        s_ones = s_mc[0:1, 384:512]

        def qk_proj_chunk(c):
            p_qk = ps_a.tile([64, TQ], F32, tag="ps_main")
            nc.tensor.matmul(
                out=p_qk, lhsT=s_wqk, rhs=s_xts[c], start=True, stop=True
            )
            # q' evac (bias applied per-partition); alternate engines
            if pick(612, 784) == "act":
                nc.scalar.activation(
                    out=s_qts[c], in_=p_qk[0:DH, :],
                    func=mybir.ActivationFunctionType.Identity, bias=s_bq,
                )
            else:
                nc.vector.tensor_scalar_add(s_qts[c], p_qk[0:DH, :], s_bq)
            if pick(612, 784) == "act":
                nc.scalar.activation(
                    out=s_kts[c], in_=p_qk[DH : 2 * DH, :],
                    func=mybir.ActivationFunctionType.Identity, bias=s_bk,
                )
            else:
                nc.vector.tensor_scalar_add(s_kts[c], p_qk[DH : 2 * DH, :], s_bk)

        # v projection for one 512-chunk (4 key tiles) in bf16, one PSUM bank,
        # ones column appended via the bias-row matmul (has_written semantics).
        def v_proj_chunk(c):
            p_v = ps_a.tile([128, 4 * (DH + 1)], F32, tag="ps_main")
            for r in range(4):
                c0 = r * (DH + 1)
                nc.tensor.matmul(
                    out=p_v[:, c0 : c0 + DH],
                    lhsT=s_xbs[c][:, r * 128 : (r + 1) * 128],
                    rhs=s_wv,
                    start=(r == 0), stop=False,
                )
                nc.tensor.matmul(
                    out=p_v[:, c0 : c0 + DH + 1], lhsT=s_ones, rhs=s_bv,
                    start=False, stop=(r == 3),
                )
            if pick(295, 262) == "act":
                nc.scalar.activation(
                    out=s_vas[c], in_=p_v,
                    func=mybir.ActivationFunctionType.Identity,
                )
            else:
                nc.vector.tensor_copy(out=s_vas[c], in_=p_v)

        def q_of(g):
            return s_qts[g]

        def k_of(j):
            return s_kts[j // 4][:, (j % 4) * 128 : (j % 4 + 1) * 128]

        def v_of(j):
            c0 = (j % 4) * (DH + 1)
            return s_vas[j // 4][:, c0 : c0 + DH + 1]

        def emit_exp(pt, p_st, width, eng):
            if eng == "act":
                nc.scalar.activation(
                    out=pt[:, 0:width], in_=p_st,
                    func=mybir.ActivationFunctionType.Exp,
                )
            else:
                nc.vector.tensor_scalar(
                    out=pt[:, 0:width].bitcast(I16), in0=p_st,
                    scalar1=A16, scalar2=B16,
                    op0=mybir.AluOpType.mult, op1=mybir.AluOpType.add,
                )

        qk_proj_chunk(0)
        qk_proj_chunk(1)

        # deferred-work FIFO: PV flushes, accumulator evacs, and group tails
        # are emitted ~LAG pair-slots behind their exp so the in-order PE
        # stream always has independent S matmuls to chew on while ACT/DVE
        # run the exps.
        work_q = []
        LAG = 2

        def drain(to_len):
            while len(work_q) > to_len:
                work_q.pop(0)()

        # Projections are emitted just-in-time as the key/value horizon grows.
        ORDER = list(range(NG))
        done_qk = {0, 1}
        done_v = set()

        def ensure_projs(kq_need, v_need):
            for c in range(max(kq_need, v_need) + 1):
                if c <= kq_need and c not in done_qk:
                    done_qk.add(c)
                    qk_proj_chunk(c)
                if c <= v_need and c not in done_v:
                    done_v.add(c)
                    v_proj_chunk(c)

        for si, g in enumerate(ORDER):
            i0 = g * TQ
            # PV accumulation: start resets the whole PSUM bank, so only the
            # group's first block starts and only its last block stops;
            # disjoint 33-col regions rely on has_written semantics.
            pv_cnt = [0]
            pv_tot = 16 * g + 10
            p_acc = ps_o.tile([128, 4 * (DH + 1)], F32, tag="ps_acc")

            def pv_block(pt_slice, j, s, p_acc=p_acc, pv_cnt=pv_cnt, pv_tot=pv_tot):
                c0 = s * (DH + 1)
                nc.tensor.matmul(
                    out=p_acc[:, c0 : c0 + DH + 1],
                    lhsT=pt_slice,
                    rhs=v_of(j),
                    start=(pv_cnt[0] == 0),
                    stop=(pv_cnt[0] + 1 == pv_tot),
                )
                pv_cnt[0] += 1

            # off-diagonal key tiles in pairs: one exp per [128, 1024]
            for q in range(2 * g):
                j0 = 2 * q
                # just-in-time projections: S of this pair needs its k chunk
                # (stay one ahead); queued PV flushes drain LAG behind and
                # need their v chunks by then. Slot 1 pre-stages the rest of
                # the qk chunks so the out-of-order groups find their own q
                # chunk ready (slot 2 runs group 7).
                kq = max((2 * q + 3) // 4 + 1, g + 1)
                ensure_projs(min(NG - 1, kq),
                             min(g, (2 * q + 2 * LAG + 1) // 4))
                p_st = ps_s.tile([128, 2 * TQ], F32, tag="ps_st")
                for u in range(2):
                    nc.tensor.matmul(
                        out=p_st[:, u * TQ : (u + 1) * TQ],
                        lhsT=k_of(j0 + u),
                        rhs=q_of(g),
                        start=True, stop=True,
                    )
                pt = pool_p.tile([128, 2 * TQ], BF16, tag="pt")
                emit_exp(pt, p_st, 2 * TQ, pick(1038, 1192))

                def pv_pair(pt=pt, j0=j0, pv_block=pv_block):
                    for u in range(2):
                        for s in range(4):
                            pv_block(pt[:, u * TQ + 128 * s : u * TQ + 128 * (s + 1)],
                                     j0 + u, s)

                work_q.append(pv_pair)
                drain(LAG)

            # diagonal tiles r=0..3 cover queries [128r, 512); causal -1e9
            # bias added by the A^T@Wm matmul on the 128 columns next to the
            # diagonal. T1 = [r0 512 | r1 384], T2 = [r2 256 | r3 128].
            ensure_projs(min(NG - 1, g + 1), g)
            p1 = ps_s.tile([128, 896], F32, tag="ps_st")
            nc.tensor.matmul(out=p1[:, 0:512], lhsT=k_of(4 * g), rhs=q_of(g),
                             start=True, stop=False)
            nc.tensor.matmul(out=p1[:, 0:128], lhsT=s_A, rhs=s_Wm,
                             start=False, stop=True)
            nc.tensor.matmul(out=p1[:, 512:896], lhsT=k_of(4 * g + 1),
                             rhs=q_of(g)[:, 128:512], start=True, stop=False)
            nc.tensor.matmul(out=p1[:, 512:640], lhsT=s_A, rhs=s_Wm,
                             start=False, stop=True)
            pt1 = pool_p.tile([128, 2 * TQ], BF16, tag="pt")
            emit_exp(pt1, p1, 896, "act")
            load["act"] += 931

            def pv_diag1(pt1=pt1, g=g, pv_block=pv_block):
                for s in range(4):
                    pv_block(pt1[:, 128 * s : 128 * (s + 1)], 4 * g, s)
                for s in range(1, 4):
                    pv_block(pt1[:, 512 + 128 * (s - 1) : 512 + 128 * s], 4 * g + 1, s)

            work_q.append(pv_diag1)
            drain(LAG)

            p2 = ps_s.tile([128, 384], F32, tag="ps_st")
            nc.tensor.matmul(out=p2[:, 0:256], lhsT=k_of(4 * g + 2),
                             rhs=q_of(g)[:, 256:512], start=True, stop=False)
            nc.tensor.matmul(out=p2[:, 0:128], lhsT=s_A, rhs=s_Wm,
                             start=False, stop=False)
            nc.tensor.matmul(out=p2[:, 256:384], lhsT=k_of(4 * g + 3),
                             rhs=q_of(g)[:, 384:512], start=False, stop=False)
            nc.tensor.matmul(out=p2[:, 256:384], lhsT=s_A, rhs=s_Wm,
                             start=False, stop=True)
            pt2 = pool_p.tile([128, 2 * TQ], BF16, tag="pt")
            emit_exp(pt2, p2, 384, "act")
            load["act"] += 505
            s_ob = pool_ot.tile([128, 4 * (DH + 1)], BF16, tag="ot")

            def pv_diag2_and_evac(pt2=pt2, g=g, s_ob=s_ob, p_acc=p_acc,
                                  pv_block=pv_block):
                for s in range(2, 4):
                    pv_block(pt2[:, 128 * (s - 2) : 128 * (s - 1)], 4 * g + 2, s)
                pv_block(pt2[:, 256:384], 4 * g + 3, 3)
                # evacuate the PV accumulator (frees psO for the next group)
                if pick(295, 262) == "act":
                    nc.scalar.activation(
                        out=s_ob, in_=p_acc,
                        func=mybir.ActivationFunctionType.Identity,
                    )
                else:
                    nc.vector.tensor_copy(out=s_ob, in_=p_acc)

            work_q.append(pv_diag2_and_evac)

            def tail(s_ob=s_ob, i0=i0):
                # transpose o [128q, 33] subtiles -> oT [33, 512] on the PE
                p_t = ps_a.tile([DH + 1, TQ], BF16, tag="ps_main")
                for s in range(4):
                    c0 = s * (DH + 1)
                    nc.tensor.matmul(
                        out=p_t[:, 128 * s : 128 * (s + 1)],
                        lhsT=s_ob[:, c0 : c0 + DH + 1],
                        rhs=s_I, is_transpose=True,
                        start=(s == 0), stop=(s == 3),
                    )
                s_ot = pool_t.tile([DH + 1, TQ], F32R, tag="oT")
                if pick(612, 783) == "act":
                    nc.scalar.activation(
                        out=s_ot, in_=p_t,
                        func=mybir.ActivationFunctionType.Identity,
                    )
                else:
                    nc.vector.tensor_copy(out=s_ot, in_=p_t)
                p_y = ps_a.tile([C, TQ], F32, tag="ps_main")
                nc.tensor.matmul(
                    out=p_y, lhsT=s_wo, rhs=s_ot[0:DH, :], start=True, stop=True
                )
                s_y = pool_y.tile([C, TQ], F32, tag="y")
                if pick(612, 783) == "act":
                    nc.scalar.activation(
                        out=s_y, in_=p_y,
                        func=mybir.ActivationFunctionType.Identity,
                    )
                else:
                    nc.vector.tensor_copy(out=s_y, in_=p_y)
                nc.sync.dma_start(out=yt[:, i0 : i0 + TQ], in_=s_y)
                nc.sync.dma_start(
                    out=sums[:, i0 : i0 + TQ],
                    in_=s_ot[DH : DH + 1, :].bitcast(F32),
                )

            work_q.append(tail)

        drain(0)

    nc.compile()
    _CACHE["nc"] = nc
    return nc


def _host_inputs(x, qkv_w, qkv_b, out_w, out_b):
    import ml_dtypes

    scale = 1.0 / math.sqrt(DH)
    mm = np.arange(128)[:, None]
    w_blk = -1e9 * (mm == np.arange(128)[None, :] + 1).astype(np.float32)
    a_blk = (mm <= np.arange(128)[None, :]).astype(np.float32)
    i_blk = np.eye(128, dtype=np.float32)
    ones_blk = np.zeros((128, 128), dtype=np.float32)
    ones_blk[0, :] = 1.0
    mconst = np.concatenate([w_blk, a_blk, i_blk, ones_blk], axis=1).astype(
        ml_dtypes.bfloat16
    )
    in_maps = []
    for c in range(NCORES):
        b, h = c // 4, c % 4
        wq = qkv_w[h * DH : (h + 1) * DH, :] * scale          # [32, 128]
        wk = qkv_w[C + h * DH : C + (h + 1) * DH, :]
        wv_ = qkv_w[2 * C + h * DH : 2 * C + (h + 1) * DH, :]
        bq = qkv_b[h * DH : (h + 1) * DH] * scale
        bk = qkv_b[C + h * DH : C + (h + 1) * DH]
        bv_ = qkv_b[2 * C + h * DH : 2 * C + (h + 1) * DH]
        wconst = np.zeros((C, 80), dtype=np.float32)
        wconst[:, 0:64] = np.concatenate([wq, wk], axis=0).T
        wconst[0:64, 64] = np.concatenate([bq, bk])
        in_maps.append(
            {
                "xt": round_fp32r(x[b].T),
                "xtb": np.ascontiguousarray(x[b].T).astype(ml_dtypes.bfloat16),
                "wconst": round_fp32r(wconst),
                "wv": wv_.T.astype(ml_dtypes.bfloat16),
                "bv": np.concatenate([bv_, [1.0]]).astype(ml_dtypes.bfloat16)[None, :],
                "wo": round_fp32r(out_w[:, h * DH : (h + 1) * DH].T),
                "mconst": np.ascontiguousarray(mconst),
            }
        )
    return in_maps


def kernel(x, qkv_w, qkv_b, out_w, out_b):
    global last_exec_time_ns, last_results
    x = np.asarray(x, dtype=np.float32)
    qkv_w = np.asarray(qkv_w, dtype=np.float32)
    qkv_b = np.asarray(qkv_b, dtype=np.float32)
    out_w = np.asarray(out_w, dtype=np.float32)
    out_b = np.asarray(out_b, dtype=np.float32)

    nc = build_program()
    in_maps = _host_inputs(x, qkv_w, qkv_b, out_w, out_b)
    try:
        res = run_bass_kernel_spmd(
            nc,
            in_maps,
            list(range(NCORES)),
            trace=bool(int(os.environ.get("KERNEL_TRACE", "0"))),
        )
    except ModuleNotFoundError:
        os.environ["BASS_NEVER_TRACE"] = "1"
        res = run_bass_kernel_spmd(nc, in_maps, list(range(NCORES)), trace=False)
    last_results = res
    last_exec_time_ns = res.exec_time_ns

    y = np.empty((B, T, C), dtype=np.float32)
    for b in range(B):
        acc = np.zeros((C, T), dtype=np.float32)
        for h in range(H):
            r = res.results[b * 4 + h]
            acc += r["yt"] / r["sums"]
        y[b] = acc.T + out_b[None, :]
    return y


# revision 29
# speedup vs baseline: 1.0499x; 1.0235x over previous
"""Causal multi-head attention block on 8 Trainium2 NeuronCores.

Problem: B=2, T=4096, C=128, H=4, Dh=32 (fp32).
  qkv = x @ qkv_w.T + qkv_b ; causal softmax attention ; y = out @ out_w.T + out_b

Sharding: 8 cores = (batch B=2) x (heads H=4). Each core owns one (b, h)
pair end to end. The device returns the *unnormalized* head output
yT[h] = (P @ V) @ Wo_h.T (transposed, [C, T]) plus the softmax row-sums;
the host divides by the row-sums, sums the 4 head partials per batch, and
adds out_b. Softmax normalization commutes with the linear maps -> exact.

On-device design (per core). The cost model charges matmuls by streamed
(moving) columns only, so P@V runs with the P^T block as the stationary
operand: o[128q, 33] += pt[128k, 128q]^T @ vaug[128k, 33] streams just 33
columns per (key-tile, query-subtile) block. pt and vaug are bf16 (full PE
rate at any width; P/v quantization ~0.4% each, well inside the 2e-2 rel
budget). o is transposed back on the PE (bf16 identity) for the output
projection; the softmax row-sum rides the PV accumulation as a ones column.

exp is the throughput limit (ACT 0.833 ns/elem/lane), so roughly half the
off-diagonal exps run on the DVE as a one-instruction Schraudolph exp:
i16 = rne_sat(s * 128/ln2 + (127*128 + c)) bitcast to bf16 gives exp(s)
within ~3.3% (final output err ~4e-3, validated in numpy + on HW). The
diagonal (causally masked, -1e9 biased) blocks stay on ACT's exact exp.
Engine assignment of exps and PSUM evacuations is a greedy static load
balance between ACT and DVE (GPSIMD cannot touch PSUM; it only triggers
SWDGE DMAs).
"""

import math
import os
from contextlib import ExitStack

import numpy as np

import concourse.bass as bass
import concourse.tile as tile
from concourse import bacc, mybir
from concourse.bass_utils import run_bass_kernel_spmd

B, T, C = 2, 4096, 128
H, DH = 4, 32
NCORES = 8
TQ = 512          # query block per group
NG = T // TQ      # 8 query groups
F32 = mybir.dt.float32
F32R = mybir.dt.float32r
BF16 = mybir.dt.bfloat16
I16 = mybir.dt.int16

# Schraudolph bf16 exp constants: i16 = rne(s*A16 + B16), bitcast bf16
A16 = 128.0 / math.log(2.0)
B16 = 127.0 * 128.0 - 5.5

_CACHE = {}
last_exec_time_ns = None
last_results = None


def round_fp32r(a):
    """Round fp32 to fp32r (drop low 12 mantissa bits, round-to-nearest-even)."""
    u = np.ascontiguousarray(a, dtype=np.float32).view(np.uint32)
    low = u & np.uint32(0xFFF)
    base = u & np.uint32(0xFFFFF000)
    up = (low > 0x800) | ((low == 0x800) & (((base >> np.uint32(12)) & np.uint32(1)) == 1))
    return (base + (up.astype(np.uint32) << np.uint32(12))).view(np.float32)


def build_program():
    if "nc" in _CACHE:
        return _CACHE["nc"]
    nc = bacc.Bacc(
        "TRN2",
        target_bir_lowering=False,
        debug=False,
        enable_asserts=False,
        num_devices=NCORES,
    )
    xt = nc.dram_tensor("xt", [C, T], F32R, kind="ExternalInput").ap()
    xtb = nc.dram_tensor("xtb", [C, T], BF16, kind="ExternalInput").ap()
    # wconst: wqk [:, 0:64] and the q'/k bias column [0:64, 64:65]
    wconst = nc.dram_tensor("wconst", [C, 80], F32R, kind="ExternalInput").ap()
    wv = nc.dram_tensor("wv", [C, DH], BF16, kind="ExternalInput").ap()
    bv = nc.dram_tensor("bv", [1, DH + 1], BF16, kind="ExternalInput").ap()
    wo = nc.dram_tensor("wo", [DH, C], F32R, kind="ExternalInput").ap()
    # mconst (bf16): Wm [128,128] (Wm[m,x] = -1e9*[m==x+1]; A^T@Wm = causal
    # -1e9 for key jj > query x), A = tril ones, I = identity, ones row.
    mconst = nc.dram_tensor("mconst", [128, 512], BF16, kind="ExternalInput").ap()
    yt = nc.dram_tensor("yt", [C, T], F32, kind="ExternalOutput").ap()
    sums = nc.dram_tensor("sums", [1, T], F32, kind="ExternalOutput").ap()

    # greedy static ACT/DVE load balance (ns estimates incl fixed overheads)
    load = {"act": 0.0, "dve": 0.0}

    def pick(act_cost, dve_cost):
        eng = "act" if load["act"] + act_cost <= load["dve"] + dve_cost else "dve"
        load[eng] += act_cost if eng == "act" else dve_cost
        return eng

    with ExitStack() as ctx:
        tc = ctx.enter_context(tile.TileContext(nc))
        const = ctx.enter_context(tc.tile_pool(name="const", bufs=1))
        pool_p = ctx.enter_context(tc.tile_pool(name="pT", bufs=6))
        pool_ot = ctx.enter_context(tc.tile_pool(name="ot", bufs=3))
        pool_t = ctx.enter_context(tc.tile_pool(name="oT", bufs=2))
        pool_y = ctx.enter_context(tc.tile_pool(name="yt", bufs=3))
        # PSUM budget (8 banks): psS 3x[128,1024] = 6 (3 S-pair slots so the
        # S(q) -> exp(q) -> S(q+3) ring never throttles the exp engines),
        # psO [128,132] = 1, psA 1 (projections + oT transpose + y rotate).
        ps_s = ctx.enter_context(tc.tile_pool(name="psS", bufs=3, space="PSUM"))
        ps_o = ctx.enter_context(tc.tile_pool(name="psO", bufs=1, space="PSUM"))
        ps_a = ctx.enter_context(tc.tile_pool(name="psA", bufs=1, space="PSUM"))

        s_wc = const.tile([C, 80], F32R)
        s_wv = const.tile([C, DH], BF16)
        s_bv = const.tile([1, DH + 1], BF16)
        s_wo = const.tile([DH, C], F32R)
        s_mc = const.tile([128, 512], BF16)
        s_xts = [const.tile([C, TQ], F32R, name=f"xt{c}") for c in range(NG)]
        s_xbs = [const.tile([C, TQ], BF16, name=f"xb{c}") for c in range(NG)]
        s_qts = [const.tile([DH, TQ], F32R, name=f"qt{c}") for c in range(NG)]
        s_kts = [const.tile([DH, TQ], F32R, name=f"kt{c}") for c in range(NG)]
        s_vas = [const.tile([128, 4 * (DH + 1)], BF16, name=f"va{c}") for c in range(NG)]

        # critical-path DMAs first, split across two parallel DMA lanes
        # (sync -> HWDGE; gpsimd -> SWDGE on the otherwise idle Q7 cores)
        nc.sync.dma_start(out=s_wc, in_=wconst)
        for c in range(4):
            nc.sync.dma_start(out=s_xts[c], in_=xt[:, c * TQ : (c + 1) * TQ])
        nc.gpsimd.dma_start(out=s_mc, in_=mconst)
        nc.gpsimd.dma_start(out=s_wv, in_=wv)
        nc.gpsimd.dma_start(out=s_bv, in_=bv)
        nc.gpsimd.dma_start(out=s_xbs[0], in_=xtb[:, 0:TQ])
        for c in range(4, NG):
            nc.gpsimd.dma_start(out=s_xts[c], in_=xt[:, c * TQ : (c + 1) * TQ])
        for c in range(1, NG):
            nc.gpsimd.dma_start(out=s_xbs[c], in_=xtb[:, c * TQ : (c + 1) * TQ])
        nc.gpsimd.dma_start(out=s_wo, in_=wo)

        s_wqk = s_wc[:, 0:64]
        s_bq = s_wc[0:DH, 64:65].bitcast(F32)
        s_bk = s_wc[DH : 2 * DH, 64:65].bitcast(F32)
        s_Wm = s_mc[:, 0:128]
        s_A = s_mc[:, 128:256]
        s_I = s_mc[:, 256:384]
        s_ones = s_mc[0:1, 384:512]

        def qk_proj_chunk(c):
            p_qk = ps_a.tile([64, TQ], F32, tag="ps_main")
            nc.tensor.matmul(
                out=p_qk, lhsT=s_wqk, rhs=s_xts[c], start=True, stop=True
            )
            # q' evac (bias applied per-partition); alternate engines
            if pick(612, 784) == "act":
                nc.scalar.activation(
                    out=s_qts[c], in_=p_qk[0:DH, :],
                    func=mybir.ActivationFunctionType.Identity, bias=s_bq,
                )
            else:
                nc.vector.tensor_scalar_add(s_qts[c], p_qk[0:DH, :], s_bq)
            if pick(612, 784) == "act":
                nc.scalar.activation(
                    out=s_kts[c], in_=p_qk[DH : 2 * DH, :],
                    func=mybir.ActivationFunctionType.Identity, bias=s_bk,
                )
            else:
                nc.vector.tensor_scalar_add(s_kts[c], p_qk[DH : 2 * DH, :], s_bk)

        # v projection for one 512-chunk (4 key tiles) in bf16, one PSUM bank,
        # ones column appended via the bias-row matmul (has_written semantics).
        def v_proj_chunk(c):
            p_v = ps_a.tile([128, 4 * (DH + 1)], F32, tag="ps_main")
            for r in range(4):
                c0 = r * (DH + 1)
                nc.tensor.matmul(
                    out=p_v[:, c0 : c0 + DH],
                    lhsT=s_xbs[c][:, r * 128 : (r + 1) * 128],
                    rhs=s_wv,
                    start=(r == 0), stop=False,
                )
                nc.tensor.matmul(
                    out=p_v[:, c0 : c0 + DH + 1], lhsT=s_ones, rhs=s_bv,
                    start=False, stop=(r == 3),
                )
            if pick(295, 262) == "act":
                nc.scalar.activation(
                    out=s_vas[c], in_=p_v,
                    func=mybir.ActivationFunctionType.Identity,
                )
            else:
                nc.vector.tensor_copy(out=s_vas[c], in_=p_v)

        def q_of(g):
            return s_qts[g]

        def k_of(j):
            return s_kts[j // 4][:, (j % 4) * 128 : (j % 4 + 1) * 128]

        def v_of(j):
            c0 = (j % 4) * (DH + 1)
            return s_vas[j // 4][:, c0 : c0 + DH + 1]

        def emit_exp(pt, p_st, width, eng):
            if eng == "act":
                nc.scalar.activation(
                    out=pt[:, 0:width], in_=p_st,
                    func=mybir.ActivationFunctionType.Exp,
                )
            else:
                nc.vector.tensor_scalar(
                    out=pt[:, 0:width].bitcast(I16), in0=p_st,
                    scalar1=A16, scalar2=B16,
                    op0=mybir.AluOpType.mult, op1=mybir.AluOpType.add,
                )

        qk_proj_chunk(0)
        qk_proj_chunk(1)

        # deferred-work FIFO: PV flushes, accumulator evacs, and group tails
        # are emitted ~LAG pair-slots behind their exp so the in-order PE
        # stream always has independent S matmuls to chew on while ACT/DVE
        # run the exps.
        work_q = []
        LAG = 2

        def drain(to_len):
            while len(work_q) > to_len:
                work_q.pop(0)()

        # Projections are emitted just-in-time as the key/value horizon grows.
        ORDER = list(range(NG))
        done_qk = {0, 1}
        done_v = set()

        def ensure_projs(kq_need, v_need):
            for c in range(max(kq_need, v_need) + 1):
                if c <= kq_need and c not in done_qk:
                    done_qk.add(c)
                    qk_proj_chunk(c)
                if c <= v_need and c not in done_v:
                    done_v.add(c)
                    v_proj_chunk(c)

        for si, g in enumerate(ORDER):
            i0 = g * TQ
            # PV accumulation: start resets the whole PSUM bank, so only the
            # group's first block starts and only its last block stops;
            # disjoint 33-col regions rely on has_written semantics.
            pv_cnt = [0]
            pv_tot = 16 * g + 10
            p_acc = ps_o.tile([128, 4 * (DH + 1)], F32, tag="ps_acc")

            def pv_block(pt_slice, j, s, p_acc=p_acc, pv_cnt=pv_cnt, pv_tot=pv_tot):
                c0 = s * (DH + 1)
                nc.tensor.matmul(
                    out=p_acc[:, c0 : c0 + DH + 1],
                    lhsT=pt_slice,
                    rhs=v_of(j),
                    start=(pv_cnt[0] == 0),
                    stop=(pv_cnt[0] + 1 == pv_tot),
                )
                pv_cnt[0] += 1

            # off-diagonal key tiles in pairs: one exp per [128, 1024]
            for q in range(2 * g):
                j0 = 2 * q
                # just-in-time projections: S of this pair needs its k chunk
                # (stay one ahead); queued PV flushes drain LAG behind and
                # need their v chunks by then. Slot 1 pre-stages the rest of
                # the qk chunks so the out-of-order groups find their own q
                # chunk ready (slot 2 runs group 7).
                if q == 0:
                    ensure_projs(min(NG - 1, g + 2), g)
                p_st = ps_s.tile([128, 2 * TQ], F32, tag="ps_st")
                for u in range(2):
                    nc.tensor.matmul(
                        out=p_st[:, u * TQ : (u + 1) * TQ],
                        lhsT=k_of(j0 + u),
                        rhs=q_of(g),
                        start=True, stop=True,
                    )
                pt = pool_p.tile([128, 2 * TQ], BF16, tag="pt")
                emit_exp(pt, p_st, 2 * TQ, pick(1038, 1192))

                def pv_pair(pt=pt, j0=j0, pv_block=pv_block):
                    for u in range(2):
                        for s in range(4):
                            pv_block(pt[:, u * TQ + 128 * s : u * TQ + 128 * (s + 1)],
                                     j0 + u, s)

                work_q.append(pv_pair)
                drain(LAG)

            # diagonal tiles r=0..3 cover queries [128r, 512); causal -1e9
            # bias added by the A^T@Wm matmul on the 128 columns next to the
            # diagonal. T1 = [r0 512 | r1 384], T2 = [r2 256 | r3 128].
            ensure_projs(min(NG - 1, g + 1), g)
            p1 = ps_s.tile([128, 896], F32, tag="ps_st")
            nc.tensor.matmul(out=p1[:, 0:512], lhsT=k_of(4 * g), rhs=q_of(g),
                             start=True, stop=False)
            nc.tensor.matmul(out=p1[:, 0:128], lhsT=s_A, rhs=s_Wm,
                             start=False, stop=True)
            nc.tensor.matmul(out=p1[:, 512:896], lhsT=k_of(4 * g + 1),
                             rhs=q_of(g)[:, 128:512], start=True, stop=False)
            nc.tensor.matmul(out=p1[:, 512:640], lhsT=s_A, rhs=s_Wm,
                             start=False, stop=True)
            pt1 = pool_p.tile([128, 2 * TQ], BF16, tag="pt")
            emit_exp(pt1, p1, 896, "act")
            load["act"] += 931

            def pv_diag1(pt1=pt1, g=g, pv_block=pv_block):
                for s in range(4):
                    pv_block(pt1[:, 128 * s : 128 * (s + 1)], 4 * g, s)
                for s in range(1, 4):
                    pv_block(pt1[:, 512 + 128 * (s - 1) : 512 + 128 * s], 4 * g + 1, s)

            work_q.append(pv_diag1)
            drain(LAG)

            p2 = ps_s.tile([128, 384], F32, tag="ps_st")
            nc.tensor.matmul(out=p2[:, 0:256], lhsT=k_of(4 * g + 2),
                             rhs=q_of(g)[:, 256:512], start=True, stop=False)
            nc.tensor.matmul(out=p2[:, 0:128], lhsT=s_A, rhs=s_Wm,
                             start=False, stop=False)
            nc.tensor.matmul(out=p2[:, 256:384], lhsT=k_of(4 * g + 3),
                             rhs=q_of(g)[:, 384:512], start=False, stop=False)
            nc.tensor.matmul(out=p2[:, 256:384], lhsT=s_A, rhs=s_Wm,
                             start=False, stop=True)
            pt2 = pool_p.tile([128, 2 * TQ], BF16, tag="pt")
            emit_exp(pt2, p2, 384, "act")
            load["act"] += 505
            s_ob = pool_ot.tile([128, 4 * (DH + 1)], BF16, tag="ot")

            def pv_diag2_and_evac(pt2=pt2, g=g, s_ob=s_ob, p_acc=p_acc,
                                  pv_block=pv_block):
                for s in range(2, 4):
                    pv_block(pt2[:, 128 * (s - 2) : 128 * (s - 1)], 4 * g + 2, s)
                pv_block(pt2[:, 256:384], 4 * g + 3, 3)
                # evacuate the PV accumulator (frees psO for the next group)
                if pick(295, 262) == "act":
                    nc.scalar.activation(
                        out=s_ob, in_=p_acc,
                        func=mybir.ActivationFunctionType.Identity,
                    )
                else:
                    nc.vector.tensor_copy(out=s_ob, in_=p_acc)

            work_q.append(pv_diag2_and_evac)

            def tail(s_ob=s_ob, i0=i0):
                # transpose o [128q, 33] subtiles -> oT [33, 512] on the PE
                p_t = ps_a.tile([DH + 1, TQ], BF16, tag="ps_main")
                for s in range(4):
                    c0 = s * (DH + 1)
                    nc.tensor.matmul(
                        out=p_t[:, 128 * s : 128 * (s + 1)],
                        lhsT=s_ob[:, c0 : c0 + DH + 1],
                        rhs=s_I, is_transpose=True,
                        start=(s == 0), stop=(s == 3),
                    )
                s_ot = pool_t.tile([DH + 1, TQ], F32R, tag="oT")
                if pick(612, 783) == "act":
                    nc.scalar.activation(
                        out=s_ot, in_=p_t,
                        func=mybir.ActivationFunctionType.Identity,
                    )
                else:
                    nc.vector.tensor_copy(out=s_ot, in_=p_t)
                p_y = ps_a.tile([C, TQ], F32, tag="ps_main")
                nc.tensor.matmul(
                    out=p_y, lhsT=s_wo, rhs=s_ot[0:DH, :], start=True, stop=True
                )
                s_y = pool_y.tile([C, TQ], F32, tag="y")
                if pick(612, 783) == "act":
                    nc.scalar.activation(
                        out=s_y, in_=p_y,
                        func=mybir.ActivationFunctionType.Identity,
                    )
                else:
                    nc.vector.tensor_copy(out=s_y, in_=p_y)
                nc.sync.dma_start(out=yt[:, i0 : i0 + TQ], in_=s_y)
                nc.sync.dma_start(
                    out=sums[:, i0 : i0 + TQ],
                    in_=s_ot[DH : DH + 1, :].bitcast(F32),
                )

            work_q.append(tail)

        drain(0)

    nc.compile()
    _CACHE["nc"] = nc
    return nc


def _host_inputs(x, qkv_w, qkv_b, out_w, out_b):
    import ml_dtypes

    scale = 1.0 / math.sqrt(DH)
    mm = np.arange(128)[:, None]
    w_blk = -1e9 * (mm == np.arange(128)[None, :] + 1).astype(np.float32)
    a_blk = (mm <= np.arange(128)[None, :]).astype(np.float32)
    i_blk = np.eye(128, dtype=np.float32)
    ones_blk = np.zeros((128, 128), dtype=np.float32)
    ones_blk[0, :] = 1.0
    mconst = np.concatenate([w_blk, a_blk, i_blk, ones_blk], axis=1).astype(
        ml_dtypes.bfloat16
    )
    in_maps = []
    for c in range(NCORES):
        b, h = c // 4, c % 4
        wq = qkv_w[h * DH : (h + 1) * DH, :] * scale          # [32, 128]
        wk = qkv_w[C + h * DH : C + (h + 1) * DH, :]
        wv_ = qkv_w[2 * C + h * DH : 2 * C + (h + 1) * DH, :]
        bq = qkv_b[h * DH : (h + 1) * DH] * scale
        bk = qkv_b[C + h * DH : C + (h + 1) * DH]
        bv_ = qkv_b[2 * C + h * DH : 2 * C + (h + 1) * DH]
        wconst = np.zeros((C, 80), dtype=np.float32)
        wconst[:, 0:64] = np.concatenate([wq, wk], axis=0).T
        wconst[0:64, 64] = np.concatenate([bq, bk])
        in_maps.append(
            {
                "xt": round_fp32r(x[b].T),
                "xtb": np.ascontiguousarray(x[b].T).astype(ml_dtypes.bfloat16),
                "wconst": round_fp32r(wconst),
                "wv": wv_.T.astype(ml_dtypes.bfloat16),
                "bv": np.concatenate([bv_, [1.0]]).astype(ml_dtypes.bfloat16)[None, :],
                "wo": round_fp32r(out_w[:, h * DH : (h + 1) * DH].T),
                "mconst": np.ascontiguousarray(mconst),
            }
        )
    return in_maps


def kernel(x, qkv_w, qkv_b, out_w, out_b):
    global last_exec_time_ns, last_results
    x = np.asarray(x, dtype=np.float32)
    qkv_w = np.asarray(qkv_w, dtype=np.float32)
    qkv_b = np.asarray(qkv_b, dtype=np.float32)
    out_w = np.asarray(out_w, dtype=np.float32)
    out_b = np.asarray(out_b, dtype=np.float32)

    nc = build_program()
    in_maps = _host_inputs(x, qkv_w, qkv_b, out_w, out_b)
    try:
        res = run_bass_kernel_spmd(
            nc,
            in_maps,
            list(range(NCORES)),
            trace=bool(int(os.environ.get("KERNEL_TRACE", "0"))),
        )
    except ModuleNotFoundError:
        os.environ["BASS_NEVER_TRACE"] = "1"
        res = run_bass_kernel_spmd(nc, in_maps, list(range(NCORES)), trace=False)
    last_results = res
    last_exec_time_ns = res.exec_time_ns

    y = np.empty((B, T, C), dtype=np.float32)
    for b in range(B):
        acc = np.zeros((C, T), dtype=np.float32)
        for h in range(H):
            r = res.results[b * 4 + h]
            acc += r["yt"] / r["sums"]
        y[b] = acc.T + out_b[None, :]
    return y


# revision 31
# speedup vs baseline: 1.0908x; 1.0389x over previous
"""Causal multi-head attention block on 8 Trainium2 NeuronCores.

Problem: B=2, T=4096, C=128, H=4, Dh=32 (fp32).
  qkv = x @ qkv_w.T + qkv_b ; causal softmax attention ; y = out @ out_w.T + out_b

Sharding: 8 cores = (batch B=2) x (heads H=4). Each core owns one (b, h)
pair end to end. The device returns the *unnormalized* head output
yT[h] = (P @ V) @ Wo_h.T (transposed, [C, T]) plus the softmax row-sums;
the host divides by the row-sums, sums the 4 head partials per batch, and
adds out_b. Softmax normalization commutes with the linear maps -> exact.

On-device design (per core). The cost model charges matmuls by streamed
(moving) columns only, so P@V runs with the P^T block as the stationary
operand: o[128q, 33] += pt[128k, 128q]^T @ vaug[128k, 33] streams just 33
columns per (key-tile, query-subtile) block. pt and vaug are bf16 (full PE
rate at any width; P/v quantization ~0.4% each, well inside the 2e-2 rel
budget). o is transposed back on the PE (bf16 identity) for the output
projection; the softmax row-sum rides the PV accumulation as a ones column.

exp is the throughput limit (ACT 0.833 ns/elem/lane), so roughly half the
off-diagonal exps run on the DVE as a one-instruction Schraudolph exp:
i16 = rne_sat(s * 128/ln2 + (127*128 + c)) bitcast to bf16 gives exp(s)
within ~3.3% (final output err ~4e-3, validated in numpy + on HW). The
diagonal (causally masked, -1e9 biased) blocks stay on ACT's exact exp.
Engine assignment of exps and PSUM evacuations is a greedy static load
balance between ACT and DVE (GPSIMD cannot touch PSUM; it only triggers
SWDGE DMAs).
"""

import math
import os
from contextlib import ExitStack

import numpy as np

import concourse.bass as bass
import concourse.tile as tile
from concourse import bacc, mybir
from concourse.bass_utils import run_bass_kernel_spmd

B, T, C = 2, 4096, 128
H, DH = 4, 32
NCORES = 8
TQ = 512          # query block per group
NG = T // TQ      # 8 query groups
F32 = mybir.dt.float32
F32R = mybir.dt.float32r
BF16 = mybir.dt.bfloat16
I16 = mybir.dt.int16

# Schraudolph bf16 exp constants: i16 = rne(s*A16 + B16), bitcast bf16
A16 = 128.0 / math.log(2.0)
B16 = 127.0 * 128.0 - 5.5

_CACHE = {}
last_exec_time_ns = None
last_results = None


def round_fp32r(a):
    """Round fp32 to fp32r (drop low 12 mantissa bits, round-to-nearest-even)."""
    u = np.ascontiguousarray(a, dtype=np.float32).view(np.uint32)
    low = u & np.uint32(0xFFF)
    base = u & np.uint32(0xFFFFF000)
    up = (low > 0x800) | ((low == 0x800) & (((base >> np.uint32(12)) & np.uint32(1)) == 1))
    return (base + (up.astype(np.uint32) << np.uint32(12))).view(np.float32)


def build_program():
    if "nc" in _CACHE:
        return _CACHE["nc"]
    nc = bacc.Bacc(
        "TRN2",
        target_bir_lowering=False,
        debug=False,
        enable_asserts=False,
        num_devices=NCORES,
    )
    xt = nc.dram_tensor("xt", [C, T], F32R, kind="ExternalInput").ap()
    xtb = nc.dram_tensor("xtb", [C, T], BF16, kind="ExternalInput").ap()
    # wconst: wqk [:, 0:64] and the q'/k bias column [0:64, 64:65]
    wconst = nc.dram_tensor("wconst", [C, 80], F32R, kind="ExternalInput").ap()
    wv = nc.dram_tensor("wv", [C, DH], BF16, kind="ExternalInput").ap()
    bv = nc.dram_tensor("bv", [1, DH + 1], BF16, kind="ExternalInput").ap()
    wo = nc.dram_tensor("wo", [DH, C], F32R, kind="ExternalInput").ap()
    # mconst (bf16): Wm [128,128] (Wm[m,x] = -1e9*[m==x+1]; A^T@Wm = causal
    # -1e9 for key jj > query x), A = tril ones, I = identity, ones row.
    mconst = nc.dram_tensor("mconst", [128, 512], BF16, kind="ExternalInput").ap()
    yt = nc.dram_tensor("yt", [C, T], F32, kind="ExternalOutput").ap()
    sums = nc.dram_tensor("sums", [1, T], F32, kind="ExternalOutput").ap()

    # greedy static ACT/DVE load balance (ns estimates incl fixed overheads)
    load = {"act": 0.0, "dve": 0.0}

    def pick(act_cost, dve_cost):
        eng = "act" if load["act"] + act_cost <= load["dve"] + dve_cost else "dve"
        load[eng] += act_cost if eng == "act" else dve_cost
        return eng

    with ExitStack() as ctx:
        tc = ctx.enter_context(tile.TileContext(nc))
        const = ctx.enter_context(tc.tile_pool(name="const", bufs=1))
        pool_p = ctx.enter_context(tc.tile_pool(name="pT", bufs=6))
        pool_ot = ctx.enter_context(tc.tile_pool(name="ot", bufs=3))
        pool_t = ctx.enter_context(tc.tile_pool(name="oT", bufs=2))
        pool_y = ctx.enter_context(tc.tile_pool(name="yt", bufs=3))
        # PSUM budget (8 banks): psS 3x[128,1024] = 6 (3 S-pair slots so the
        # S(q) -> exp(q) -> S(q+3) ring never throttles the exp engines),
        # psO [128,132] = 1, psA 1 (projections + oT transpose + y rotate).
        ps_s = ctx.enter_context(tc.tile_pool(name="psS", bufs=3, space="PSUM"))
        ps_o = ctx.enter_context(tc.tile_pool(name="psO", bufs=1, space="PSUM"))
        ps_a = ctx.enter_context(tc.tile_pool(name="psA", bufs=1, space="PSUM"))

        s_wc = const.tile([C, 80], F32R)
        s_wv = const.tile([C, DH], BF16)
        s_bv = const.tile([1, DH + 1], BF16)
        s_wo = const.tile([DH, C], F32R)
        s_mc = const.tile([128, 512], BF16)
        s_xts = [const.tile([C, TQ], F32R, name=f"xt{c}") for c in range(NG)]
        s_xbs = [const.tile([C, TQ], BF16, name=f"xb{c}") for c in range(NG)]
        s_qts = [const.tile([DH, TQ], F32R, name=f"qt{c}") for c in range(NG)]
        s_kts = [const.tile([DH, TQ], F32R, name=f"kt{c}") for c in range(NG)]
        s_vas = [const.tile([128, 4 * (DH + 1)], BF16, name=f"va{c}") for c in range(NG)]

        # critical-path DMAs first, split across two parallel DMA lanes
        # (sync -> HWDGE; gpsimd -> SWDGE on the otherwise idle Q7 cores)
        nc.sync.dma_start(out=s_wc, in_=wconst)
        for c in range(4):
            nc.sync.dma_start(out=s_xts[c], in_=xt[:, c * TQ : (c + 1) * TQ])
        nc.gpsimd.dma_start(out=s_mc, in_=mconst)
        nc.gpsimd.dma_start(out=s_wv, in_=wv)
        nc.gpsimd.dma_start(out=s_bv, in_=bv)
        nc.gpsimd.dma_start(out=s_xbs[0], in_=xtb[:, 0:TQ])
        for c in range(4, NG):
            nc.gpsimd.dma_start(out=s_xts[c], in_=xt[:, c * TQ : (c + 1) * TQ])
        for c in range(1, NG):
            nc.gpsimd.dma_start(out=s_xbs[c], in_=xtb[:, c * TQ : (c + 1) * TQ])
        nc.gpsimd.dma_start(out=s_wo, in_=wo)

        s_wqk = s_wc[:, 0:64]
        s_bq = s_wc[0:DH, 64:65].bitcast(F32)
        s_bk = s_wc[DH : 2 * DH, 64:65].bitcast(F32)
        s_Wm = s_mc[:, 0:128]
        s_A = s_mc[:, 128:256]
        s_I = s_mc[:, 256:384]
        s_ones = s_mc[0:1, 384:512]

        def qk_proj_chunk(c):
            p_qk = ps_a.tile([64, TQ], F32, tag="ps_main")
            nc.tensor.matmul(
                out=p_qk, lhsT=s_wqk, rhs=s_xts[c], start=True, stop=True
            )
            # q' evac (bias applied per-partition); alternate engines
            if pick(612, 784) == "act":
                nc.scalar.activation(
                    out=s_qts[c], in_=p_qk[0:DH, :],
                    func=mybir.ActivationFunctionType.Identity, bias=s_bq,
                )
            else:
                nc.vector.tensor_scalar_add(s_qts[c], p_qk[0:DH, :], s_bq)
            if pick(612, 784) == "act":
                nc.scalar.activation(
                    out=s_kts[c], in_=p_qk[DH : 2 * DH, :],
                    func=mybir.ActivationFunctionType.Identity, bias=s_bk,
                )
            else:
                nc.vector.tensor_scalar_add(s_kts[c], p_qk[DH : 2 * DH, :], s_bk)

        # v projection for one 512-chunk (4 key tiles) in bf16, one PSUM bank,
        # ones column appended via the bias-row matmul (has_written semantics).
        def v_proj_chunk(c):
            p_v = ps_a.tile([128, 4 * (DH + 1)], F32, tag="ps_main")
            for r in range(4):
                c0 = r * (DH + 1)
                nc.tensor.matmul(
                    out=p_v[:, c0 : c0 + DH],
                    lhsT=s_xbs[c][:, r * 128 : (r + 1) * 128],
                    rhs=s_wv,
                    start=(r == 0), stop=False,
                )
                nc.tensor.matmul(
                    out=p_v[:, c0 : c0 + DH + 1], lhsT=s_ones, rhs=s_bv,
                    start=False, stop=(r == 3),
                )
            if pick(295, 262) == "act":
                nc.scalar.activation(
                    out=s_vas[c], in_=p_v,
                    func=mybir.ActivationFunctionType.Identity,
                )
            else:
                nc.vector.tensor_copy(out=s_vas[c], in_=p_v)

        def q_of(g):
            return s_qts[g]

        def k_of(j):
            return s_kts[j // 4][:, (j % 4) * 128 : (j % 4 + 1) * 128]

        def v_of(j):
            c0 = (j % 4) * (DH + 1)
            return s_vas[j // 4][:, c0 : c0 + DH + 1]

        def emit_exp(pt, p_st, width, eng):
            if eng == "act":
                nc.scalar.activation(
                    out=pt[:, 0:width], in_=p_st,
                    func=mybir.ActivationFunctionType.Exp,
                )
            else:
                nc.vector.tensor_scalar(
                    out=pt[:, 0:width].bitcast(I16), in0=p_st,
                    scalar1=A16, scalar2=B16,
                    op0=mybir.AluOpType.mult, op1=mybir.AluOpType.add,
                )

        qk_proj_chunk(0)
        qk_proj_chunk(1)

        # deferred-work FIFO: PV flushes, accumulator evacs, and group tails
        # are emitted ~LAG pair-slots behind their exp so the in-order PE
        # stream always has independent S matmuls to chew on while ACT/DVE
        # run the exps.
        work_q = []
        LAG = 2

        def drain(to_len):
            while len(work_q) > to_len:
                work_q.pop(0)()

        # Projections are emitted just-in-time as the key/value horizon grows.
        ORDER = list(range(NG))
        done_qk = {0, 1}
        done_v = set()

        def ensure_projs(kq_need, v_need):
            for c in range(max(kq_need, v_need) + 1):
                if c <= kq_need and c not in done_qk:
                    done_qk.add(c)
                    qk_proj_chunk(c)
                if c <= v_need and c not in done_v:
                    done_v.add(c)
                    v_proj_chunk(c)

        for si, g in enumerate(ORDER):
            i0 = g * TQ
            # PV accumulation: start resets the whole PSUM bank, so only the
            # group's first block starts and only its last block stops;
            # disjoint 33-col regions rely on has_written semantics.
            pv_cnt = [0]
            pv_tot = 16 * g + 10
            p_acc = ps_o.tile([128, 4 * (DH + 1)], F32, tag="ps_acc")

            def pv_block(pt_slice, j, s, p_acc=p_acc, pv_cnt=pv_cnt, pv_tot=pv_tot):
                c0 = s * (DH + 1)
                nc.tensor.matmul(
                    out=p_acc[:, c0 : c0 + DH + 1],
                    lhsT=pt_slice,
                    rhs=v_of(j),
                    start=(pv_cnt[0] == 0),
                    stop=(pv_cnt[0] + 1 == pv_tot),
                )
                pv_cnt[0] += 1

            # off-diagonal key tiles in pairs: one exp per [128, 1024]
            for q in range(2 * g):
                j0 = 2 * q
                # just-in-time projections: S of this pair needs its k chunk
                # (stay one ahead); queued PV flushes drain LAG behind and
                # need their v chunks by then. Slot 1 pre-stages the rest of
                # the qk chunks so the out-of-order groups find their own q
                # chunk ready (slot 2 runs group 7).
                if q == 0:
                    ensure_projs(min(NG - 1, g + 2), g)
                p_st = ps_s.tile([128, 2 * TQ], F32, tag="ps_st")
                for u in range(2):
                    nc.tensor.matmul(
                        out=p_st[:, u * TQ : (u + 1) * TQ],
                        lhsT=k_of(j0 + u),
                        rhs=q_of(g),
                        start=True, stop=True,
                    )
                pt = pool_p.tile([128, 2 * TQ], BF16, tag="pt")
                emit_exp(pt, p_st, 2 * TQ, pick(1038, 1192))

                def pv_pair(pt=pt, j0=j0, pv_block=pv_block):
                    for u in range(2):
                        for s in range(4):
                            pv_block(pt[:, u * TQ + 128 * s : u * TQ + 128 * (s + 1)],
                                     j0 + u, s)

                work_q.append(pv_pair)
                drain(LAG)

            # diagonal tiles r=0..3 cover queries [128r, 512); causal -1e9
            # bias added by the A^T@Wm matmul on the 128 columns next to the
            # diagonal. T1 = [r0 512 | r1 384], T2 = [r2 256 | r3 128].
            p1 = ps_s.tile([128, 896], F32, tag="ps_st")
            nc.tensor.matmul(out=p1[:, 0:512], lhsT=k_of(4 * g), rhs=q_of(g),
                             start=True, stop=False)
            nc.tensor.matmul(out=p1[:, 0:128], lhsT=s_A, rhs=s_Wm,
                             start=False, stop=True)
            nc.tensor.matmul(out=p1[:, 512:896], lhsT=k_of(4 * g + 1),
                             rhs=q_of(g)[:, 128:512], start=True, stop=False)
            nc.tensor.matmul(out=p1[:, 512:640], lhsT=s_A, rhs=s_Wm,
                             start=False, stop=True)
            pt1 = pool_p.tile([128, 2 * TQ], BF16, tag="pt")
            emit_exp(pt1, p1, 896, "act")
            load["act"] += 931
            ensure_projs(min(NG - 1, g + 2), g)

            def pv_diag1(pt1=pt1, g=g, pv_block=pv_block):
                for s in range(4):
                    pv_block(pt1[:, 128 * s : 128 * (s + 1)], 4 * g, s)
                for s in range(1, 4):
                    pv_block(pt1[:, 512 + 128 * (s - 1) : 512 + 128 * s], 4 * g + 1, s)

            work_q.append(pv_diag1)
            drain(LAG)

            p2 = ps_s.tile([128, 384], F32, tag="ps_st")
            nc.tensor.matmul(out=p2[:, 0:256], lhsT=k_of(4 * g + 2),
                             rhs=q_of(g)[:, 256:512], start=True, stop=False)
            nc.tensor.matmul(out=p2[:, 0:128], lhsT=s_A, rhs=s_Wm,
                             start=False, stop=False)
            nc.tensor.matmul(out=p2[:, 256:384], lhsT=k_of(4 * g + 3),
                             rhs=q_of(g)[:, 384:512], start=False, stop=False)
            nc.tensor.matmul(out=p2[:, 256:384], lhsT=s_A, rhs=s_Wm,
                             start=False, stop=True)
            pt2 = pool_p.tile([128, 2 * TQ], BF16, tag="pt")
            emit_exp(pt2, p2, 384, "act")
            load["act"] += 505
            s_ob = pool_ot.tile([128, 4 * (DH + 1)], BF16, tag="ot")

            def pv_diag2_and_evac(pt2=pt2, g=g, s_ob=s_ob, p_acc=p_acc,
                                  pv_block=pv_block):
                for s in range(2, 4):
                    pv_block(pt2[:, 128 * (s - 2) : 128 * (s - 1)], 4 * g + 2, s)
                pv_block(pt2[:, 256:384], 4 * g + 3, 3)
                # evacuate the PV accumulator (frees psO for the next group)
                if pick(295, 262) == "act":
                    nc.scalar.activation(
                        out=s_ob, in_=p_acc,
                        func=mybir.ActivationFunctionType.Identity,
                    )
                else:
                    nc.vector.tensor_copy(out=s_ob, in_=p_acc)

            work_q.append(pv_diag2_and_evac)

            def tail(s_ob=s_ob, i0=i0):
                # transpose o [128q, 33] subtiles -> oT [33, 512] on the PE
                p_t = ps_a.tile([DH + 1, TQ], BF16, tag="ps_main")
                for s in range(4):
                    c0 = s * (DH + 1)
                    nc.tensor.matmul(
                        out=p_t[:, 128 * s : 128 * (s + 1)],
                        lhsT=s_ob[:, c0 : c0 + DH + 1],
                        rhs=s_I, is_transpose=True,
                        start=(s == 0), stop=(s == 3),
                    )
                s_ot = pool_t.tile([DH + 1, TQ], F32R, tag="oT")
                if pick(612, 783) == "act":
                    nc.scalar.activation(
                        out=s_ot, in_=p_t,
                        func=mybir.ActivationFunctionType.Identity,
                    )
                else:
                    nc.vector.tensor_copy(out=s_ot, in_=p_t)
                p_y = ps_a.tile([C, TQ], F32, tag="ps_main")
                nc.tensor.matmul(
                    out=p_y, lhsT=s_wo, rhs=s_ot[0:DH, :], start=True, stop=True
                )
                s_y = pool_y.tile([C, TQ], F32, tag="y")
                if pick(612, 783) == "act":
                    nc.scalar.activation(
                        out=s_y, in_=p_y,
                        func=mybir.ActivationFunctionType.Identity,
                    )
                else:
                    nc.vector.tensor_copy(out=s_y, in_=p_y)
                nc.sync.dma_start(out=yt[:, i0 : i0 + TQ], in_=s_y)
                nc.sync.dma_start(
                    out=sums[:, i0 : i0 + TQ],
                    in_=s_ot[DH : DH + 1, :].bitcast(F32),
                )

            work_q.append(tail)

        drain(0)

    nc.compile()
    _CACHE["nc"] = nc
    return nc


def _host_inputs(x, qkv_w, qkv_b, out_w, out_b):
    import ml_dtypes

    scale = 1.0 / math.sqrt(DH)
    mm = np.arange(128)[:, None]
    w_blk = -1e9 * (mm == np.arange(128)[None, :] + 1).astype(np.float32)
    a_blk = (mm <= np.arange(128)[None, :]).astype(np.float32)
    i_blk = np.eye(128, dtype=np.float32)
    ones_blk = np.zeros((128, 128), dtype=np.float32)
    ones_blk[0, :] = 1.0
    mconst = np.concatenate([w_blk, a_blk, i_blk, ones_blk], axis=1).astype(
        ml_dtypes.bfloat16
    )
    in_maps = []
    for c in range(NCORES):
        b, h = c // 4, c % 4
        wq = qkv_w[h * DH : (h + 1) * DH, :] * scale          # [32, 128]
        wk = qkv_w[C + h * DH : C + (h + 1) * DH, :]
        wv_ = qkv_w[2 * C + h * DH : 2 * C + (h + 1) * DH, :]
        bq = qkv_b[h * DH : (h + 1) * DH] * scale
        bk = qkv_b[C + h * DH : C + (h + 1) * DH]
        bv_ = qkv_b[2 * C + h * DH : 2 * C + (h + 1) * DH]
        wconst = np.zeros((C, 80), dtype=np.float32)
        wconst[:, 0:64] = np.concatenate([wq, wk], axis=0).T
        wconst[0:64, 64] = np.concatenate([bq, bk])
        in_maps.append(
            {
                "xt": round_fp32r(x[b].T),
                "xtb": np.ascontiguousarray(x[b].T).astype(ml_dtypes.bfloat16),
                "wconst": round_fp32r(wconst),
                "wv": wv_.T.astype(ml_dtypes.bfloat16),
                "bv": np.concatenate([bv_, [1.0]]).astype(ml_dtypes.bfloat16)[None, :],
                "wo": round_fp32r(out_w[:, h * DH : (h + 1) * DH].T),
                "mconst": np.ascontiguousarray(mconst),
            }
        )
    return in_maps


def kernel(x, qkv_w, qkv_b, out_w, out_b):
    global last_exec_time_ns, last_results
    x = np.asarray(x, dtype=np.float32)
    qkv_w = np.asarray(qkv_w, dtype=np.float32)
    qkv_b = np.asarray(qkv_b, dtype=np.float32)
    out_w = np.asarray(out_w, dtype=np.float32)
    out_b = np.asarray(out_b, dtype=np.float32)

    nc = build_program()
    in_maps = _host_inputs(x, qkv_w, qkv_b, out_w, out_b)
    try:
        res = run_bass_kernel_spmd(
            nc,
            in_maps,
            list(range(NCORES)),
            trace=bool(int(os.environ.get("KERNEL_TRACE", "0"))),
        )
    except ModuleNotFoundError:
        os.environ["BASS_NEVER_TRACE"] = "1"
        res = run_bass_kernel_spmd(nc, in_maps, list(range(NCORES)), trace=False)
    last_results = res
    last_exec_time_ns = res.exec_time_ns

    y = np.empty((B, T, C), dtype=np.float32)
    for b in range(B):
        acc = np.zeros((C, T), dtype=np.float32)
        for h in range(H):
            r = res.results[b * 4 + h]
            acc += r["yt"] / r["sums"]
        y[b] = acc.T + out_b[None, :]
    return y
